# revision 12
# baseline (speedup 1.0000x reference)
"""Trainium2 Bass kernel for nn_AutoencoderHybrid (12-qubit QAE hybrid).

Math: the circuit measures Z on wires 0..3 only. The CNOT chain (i -> i+1)
propagates information forward only, so in the Heisenberg picture each
observable Z_w pulled back through the 2-layer circuit is supported on wires
0..w+1 (at most wires 0..4). With the product input state
|psi_b> = kron_j [cos(x_j/2), -i sin(x_j/2)], the diagonal phase factors
D = diag((-i)^popcount) fold into the observable, leaving a REAL quadratic
form on a real product vector. Moreover S_w = Stil_w (x) I_{2^(3-w)} — the
quadform for observable w contracts over only d_w = 2^(w+2) dims:

    latent_w(b) = v_w^T Stil_w v_w,  v_w = kron_{j<=w+1} [cos(x_j/2), sin(x_j/2)]

The prefix vectors A(4) = v_0, B(8) = v_1, D(16) = v_2, R(32) = v_3 all fall
out of the same kron tree. Device computes per batch row the concat vector
vcat = [R|D|B|A] (60 slots, 64-padded) per group, one PE matmul against the
block-diagonal prefix matrix (240 cols vs 512 for the dense form), an
elementwise multiply + ragged grouped reductions, then the tiny decoder MLP
in transposed space (b1 folded into the ACT relu bias, b2 into the final
PSUM->SBUF add).

S_w / MLP weights are tiny (depend only on q_params etc.) and are prepared on
the host; all batch-dim work (B = 8192) runs on 8 NeuronCores, data parallel,
1024 rows per core laid out as b = 8p + c (p partition, c free-dim group).

Scheduling: the x DMA instruction is hoisted into the entry block BEFORE the
all-engine start barrier (saves ~700ns of DMA pipeline latency); constants
arrive as two packed DMAs right behind it on the SP queue; the two output
halves are DMA'd out independently (SP + ACT queues) as soon as each is
written.
"""
import math
import numpy as np

N5 = 5
NLAYERS = 2
LATENT = 4
B = 8192
NCORES = 8
BLOC = B // NCORES  # 1024

# slot layout per group inside vcat (64-wide): [R:0..32|D:32..48|B:48..56|A:56..60]
_SLOT_OFF = {3: 0, 2: 32, 1: 48, 0: 56}
_SLOT_DIM = {3: 32, 2: 16, 1: 8, 0: 4}

# ----------------------------------------------------------------------------
# Host-side constant construction (pure numpy)
# ----------------------------------------------------------------------------


def _rot(phi, theta, omega):
    c, s = np.cos(theta / 2), np.sin(theta / 2)
    ep = np.exp(-0.5j * (phi + omega))
    em = np.exp(-0.5j * (phi - omega))
    return np.array([[ep * c, -np.conj(em) * s], [em * s, np.conj(ep) * c]],
                    dtype=np.complex128)


def _build_S(q_params):
    """(4, 32, 32) real symmetric: latent_w = r^T S_w r (unit-norm slots)."""
    qp = np.asarray(q_params, np.float64)
    dim = 2 ** N5
    eye2 = np.eye(2)

    def kron_at(U, wire):
        M = np.array([[1.0]])
        for j in range(N5):
            M = np.kron(M, U if j == wire else eye2)
        return M

    def cnot_mat(c, t):
        M = np.zeros((dim, dim))
        for z in range(dim):
            bits = [(z >> (N5 - 1 - j)) & 1 for j in range(N5)]
            if bits[c] == 1:
                bits[t] ^= 1
            z2 = 0
            for b in bits:
                z2 = (z2 << 1) | b
            M[z2, z] = 1.0
        return M

    V = np.eye(dim, dtype=np.complex128)
    for l in range(NLAYERS):
        for i in range(N5):
            V = kron_at(_rot(*qp[l, i]), i) @ V
        for i in range(N5 - 1):
            V = cnot_mat(i, i + 1) @ V

    pc = np.array([bin(z).count("1") for z in range(dim)])
    D = np.diag((-1j) ** pc)
    VD = V @ D
    Ss = []
    for w in range(LATENT):
        zdiag = np.array([1.0 if ((z >> (N5 - 1 - w)) & 1) == 0 else -1.0
                          for z in range(dim)])
        O = VD.conj().T @ (zdiag[:, None] * VD)
        Ss.append(np.real(O))
    return np.stack(Ss)


def _host_consts(q_params, W1, b1, W2, b2):
    S = _build_S(q_params)  # (4, 32, 32), unscaled
    # prefix block-diagonal quadform matrix, device slots carry cos/2, sin/2
    # so fold 4^(w+2) per observable
    M1 = np.zeros((64, 60))
    for w in range(4):
        d = _SLOT_DIM[w]
        o = _SLOT_OFF[w]
        tail = 32 // d
        M1[o:o + d, o:o + d] = S[w][::tail, ::tail] * (4.0 ** (w + 2))
    mproj = np.zeros((128, 120), np.float32)
    mproj[0:64, 0:60] = M1
    mproj[64:128, 60:120] = M1

    W1 = np.asarray(W1, np.float64)
    b1 = np.asarray(b1, np.float64)
    W2 = np.asarray(W2, np.float64)
    b2 = np.asarray(b2, np.float64)

    # lat layout [p, 4*ws + g], ws order [w3, w2, w1, w0]
    w1t = np.zeros((16, 128), np.float32)
    for ws in range(4):
        w = 3 - ws
        for g in range(4):
            w1t[4 * ws + g, 32 * g:32 * g + 32] = W1[:, w]
    w2blk = np.zeros((128, 48), np.float32)
    for g in range(4):
        w2blk[32 * g:32 * g + 32, 12 * g:12 * g + 12] = W2.T

    # PE consts: [mproj(0:120) | w2blk(120:168) | w1t(168:296 rows 0:16)]
    pecst = np.zeros((128, 296), np.float32)
    pecst[:, 0:120] = mproj
    pecst[:, 120:168] = w2blk
    pecst[0:16, 168:296] = w1t

    # vector consts: [b2rep(0:48) | b1T(48)]
    vcst = np.zeros((128, 52), np.float32)
    vcst[:, 0:48] = np.tile(b2, 4)[None, :]
    vcst[:, 48] = np.tile(b1, 4)
    return dict(pecst=pecst, vcst=vcst)


# ----------------------------------------------------------------------------
# Device kernel body (Bass/Tile)
# ----------------------------------------------------------------------------


def _build_body(ctx, tc, x, pecst, vcst, y):
    import concourse.bass as bass
    from concourse import mybir
    nc = tc.nc
    f32 = mybir.dt.float32
    f32r = mybir.dt.float32r
    AF = mybir.ActivationFunctionType
    AX = mybir.AxisListType
    ALU = mybir.AluOpType

    def fv(t, col, dims):
        """View of tile t at free-offset col with custom free dims."""
        return bass.AP(tensor=t.tensor, offset=t.offset + col,
                       ap=[list(t.ap[0])] + [list(d) for d in dims])

    consts = ctx.enter_context(tc.tile_pool(name="consts", bufs=1))
    sb = ctx.enter_context(tc.tile_pool(name="sb", bufs=1))
    sb2 = ctx.enter_context(tc.tile_pool(name="sb2", bufs=2))
    ps = ctx.enter_context(tc.tile_pool(name="ps", bufs=2, space="PSUM"))

    # ---- x load: hoisted before the entry barrier post-schedule (SP queue)
    x_s = sb.tile([128, 96], f32)
    xa = bass.AP(tensor=x.tensor, offset=0, ap=[[96, 128], [1, 96]])
    xdma = nc.sync.dma_start(x_s[:, :], xa)

    # ---- packed constants, also SP queue (issues right behind x)
    pe_s = consts.tile([128, 296], f32r)
    nc.sync.dma_start(pe_s[:, :], pecst.bitcast(f32r))
    v_s = consts.tile([128, 52], f32)
    nc.sync.dma_start(v_s[:, :], vcst)
    mproj_v = pe_s[:, 0:120]
    w2_v = pe_s[:, 120:168]
    w1t_v = pe_s[0:16, 168:296]
    b2_v = v_s[:, 0:48]
    b1_v = v_s[:, 48:49]

    bias_c = consts.tile([128, 1], f32)
    nc.vector.memset(bias_c[:, :], math.pi / 2)
    ones128 = consts.tile([128, 128], f32)
    nc.vector.memset(ones128[:, :], 1.0)
    ident = consts.tile([128, 128], f32)
    nc.gpsimd.affine_select(out=ident[:, :], in_=ones128[:, :],
                            pattern=[[1, 128]],
                            compare_op=ALU.is_equal, fill=0.0,
                            base=0, channel_multiplier=-1)

    # ---- quarter angle: s4 = sin(x/4), c4 = cos(x/4) = sin(x/4 + pi/2)
    # both halves in one op each: sc4[p, 10c + j] = s4, 10c + 5 + j = c4
    sc4 = sb.tile([128, 80], f32)
    xin = fv(x_s, 0, [[12, 8], [1, 5]])
    nc.scalar.activation(fv(sc4, 0, [[10, 8], [1, 5]]), xin, AF.Sin,
                         bias=0.0, scale=0.25)
    nc.scalar.activation(fv(sc4, 5, [[10, 8], [1, 5]]), xin, AF.Sin,
                         bias=bias_c[:, 0:1], scale=0.25)

    # ---- per-half slot assembly + prefix kron tree into vcat
    # cs[p, 10c + j] = cos(x_j/2)/2 = c4^2 - 1/2 ; 10c+5+j = sin(x_j/2)/2
    # vcat per group g (64-aligned): [R(32) | D(16) | B(8) | A(4) | pad]
    vch, csh = [], []
    for h in (0, 1):
        E = nc.vector if h == 0 else nc.gpsimd
        sqt = sb.tile([128, 20], f32, name=f"sq{h}")
        cst = sb.tile([128, 40], f32, name=f"cs{h}")
        E.tensor_mul(fv(sqt, 0, [[5, 4], [1, 5]]),
                     fv(sc4, 40 * h + 5, [[10, 4], [1, 5]]),
                     fv(sc4, 40 * h + 5, [[10, 4], [1, 5]]))
        E.tensor_mul(fv(cst, 5, [[10, 4], [1, 5]]),
                     fv(sc4, 40 * h, [[10, 4], [1, 5]]),
                     fv(sc4, 40 * h + 5, [[10, 4], [1, 5]]))
        E.tensor_scalar_sub(fv(cst, 0, [[10, 4], [1, 5]]),
                            fv(sqt, 0, [[5, 4], [1, 5]]), 0.5)
        vc = sb.tile([128, 256], f32, name=f"vc{h}")
        # A[64g + 56 + 2 z0 + z1] = u0[z0] * u1[z1]
        E.tensor_mul(fv(vc, 56, [[64, 4], [2, 2], [1, 2]]),
                     fv(cst, 1, [[10, 4], [0, 2], [5, 2]]),
                     fv(cst, 0, [[10, 4], [5, 2], [0, 2]]))
        # B[64g + 48 + 2 a + z2] = A[a] * u2[z2]
        E.tensor_mul(fv(vc, 48, [[64, 4], [2, 4], [1, 2]]),
                     fv(vc, 56, [[64, 4], [1, 4], [0, 2]]),
                     fv(cst, 2, [[10, 4], [0, 4], [5, 2]]))
        # D[64g + 32 + 2 b + z3] = B[b] * u3[z3]
        E.tensor_mul(fv(vc, 32, [[64, 4], [2, 8], [1, 2]]),
                     fv(vc, 48, [[64, 4], [1, 8], [0, 2]]),
                     fv(cst, 3, [[10, 4], [0, 8], [5, 2]]))
        # R[64g + 2 d + z4] = D[d] * u4[z4]
        E.tensor_mul(fv(vc, 0, [[64, 4], [2, 16], [1, 2]]),
                     fv(vc, 32, [[64, 4], [1, 16], [0, 2]]),
                     fv(cst, 4, [[10, 4], [0, 16], [5, 2]]))
        vch.append(vc)
        csh.append(cst)

    # ---- PE transposes of vcat chunks (2 per half), PSUM->SBUF copies
    vT = {}
    tps = {}
    for h in (0, 1):
        for c in (0, 1):
            tp = ps.tile([128, 128], f32, tag="tp", bufs=4)
            nc.tensor.transpose(tp[:, :], vch[h][:, 128 * c:128 * c + 128],
                                ident[:, :])
            tps[(h, c)] = tp
    for h in (0, 1):
        for c in (0, 1):
            # GPSIMD cannot read PSUM: DVE takes chunk 0, ACT chunk 1
            vs = sb2.tile([128, 128], f32r, tag=f"vT{h}{c}", bufs=1)
            if c == 0:
                nc.vector.tensor_copy(vs[:, :], tps[(h, c)][:, :])
            else:
                nc.scalar.copy(vs[:, :], tps[(h, c)][:, :])
            vT[(h, c)] = vs

    # ---- quadform: Y = vcatT.T @ mproj per chunk; Pm = Y * vcat slots
    # both halves share one PSUM bank: h0 cols 0:240, h1 cols 240:480
    ybank = ps.tile([128, 480], f32, tag="Y", bufs=1)
    Y4 = {}
    for h in (0, 1):
        for c in (0, 1):
            o = 240 * h + 120 * c
            nc.tensor.matmul(ybank[:, o:o + 120], lhsT=vT[(h, c)][:, :],
                             rhs=mproj_v, start=True, stop=True)
        Y4[h] = ybank[:, 240 * h:240 * h + 240]

    Pm = {}
    lat_all = sb.tile([128, 32], f32)
    for h in (0, 1):
        pm = sb.tile([128, 240], f32, name=f"Pm{h}")
        nc.vector.tensor_mul(fv(pm, 0, [[60, 4], [1, 60]]),
                             fv(ybank, 240 * h, [[60, 4], [1, 60]]),
                             fv(vch[h], 0, [[64, 4], [1, 60]]))
        Pm[h] = pm

    # ragged grouped reductions (DVE only): lat[p, 16h + 4 ws + g]
    for h in (0, 1):
        for ws in range(4):
            w = 3 - ws
            o, d = _SLOT_OFF[w], _SLOT_DIM[w]
            nc.vector.reduce_sum(fv(lat_all, 16 * h + 4 * ws, [[1, 4]]),
                                 fv(Pm[h], o, [[60, 4], [1, d]]),
                                 axis=AX.X)

    # ---- MLP in transposed space (shared PSUM banks across halves)
    latT_p = ps.tile([16, 256], f32, tag="latT", bufs=1)
    hT_p = ps.tile([128, 256], f32, tag="hT", bufs=1)
    y4_p = ps.tile([128, 96], f32, tag="y4", bufs=1)
    y_s = sb.tile([128, 96], f32)
    for h in (0, 1):
        nc.tensor.transpose(latT_p[:, 128 * h:128 * h + 128],
                            lat_all[:, 16 * h:16 * h + 16], ident[:, :])
        latT_s = sb2.tile([16, 128], f32r, tag=f"latTs{h}", bufs=1)
        if h == 0:
            nc.vector.tensor_copy(latT_s[:, :], latT_p[:, 0:128])
        else:
            nc.scalar.copy(latT_s[:, :], latT_p[:, 128:256])
        nc.tensor.matmul(hT_p[:, 128 * h:128 * h + 128], lhsT=w1t_v,
                         rhs=latT_s[:, :], start=True, stop=True)
        # relu with b1 folded in as the per-partition ACT bias
        hT_s = sb2.tile([128, 128], f32r, tag=f"hTs{h}", bufs=1)
        nc.scalar.activation(hT_s[:, :], hT_p[:, 128 * h:128 * h + 128],
                             AF.Relu, bias=b1_v, scale=1.0)
        nc.tensor.matmul(y4_p[:, 48 * h:48 * h + 48], lhsT=hT_s[:, :],
                         rhs=w2_v, start=True, stop=True)
        # b2 add doubles as the PSUM->SBUF copy
        nc.vector.tensor_add(fv(y_s, 48 * h, [[1, 48]]),
                             fv(y4_p, 48 * h, [[1, 48]]), b2_v)
        # per-half output DMA, SP queue for h0, ACT queue for h1
        ya = bass.AP(tensor=y.tensor, offset=48 * h, ap=[[96, 128], [1, 48]])
        Q = nc.sync if h == 0 else nc.scalar
        Q.dma_start(ya, y_s[:, 48 * h:48 * h + 48])

    return xdma


def _hoist_pre_barrier(nc, inst):
    """Move `inst` (a BassInstruction) into the entry block before the first
    SP-engine instruction (i.e. before the all-engine start barrier)."""
    from concourse import mybir
    ins = inst.ins
    fn = nc.m.functions[0]
    blocks = fn.blocks
    src = None
    for b in blocks:
        for i2 in b.instructions:
            if i2.name == ins.name:
                src = b
                break
        if src is not None:
            break
    assert src is not None, "hoist: dma instruction not found"
    entry = blocks[0]
    src.instructions.remove(ins)
    idx = 0
    for k, i2 in enumerate(entry.instructions):
        if i2.engine == mybir.EngineType.SP:
            idx = k
            break
    entry.instructions.insert(idx, ins)


_NC_CACHE = {}


def _get_nc():
    if "nc" in _NC_CACHE:
        return _NC_CACHE["nc"]
    from contextlib import ExitStack
    import concourse.bacc as bacc
    import concourse.tile as tile
    from concourse import mybir
    f32 = mybir.dt.float32
    nc = bacc.Bacc("TRN2", target_bir_lowering=False, debug=False)
    x = nc.dram_tensor("x", [BLOC, 12], f32, kind="ExternalInput").ap()
    pecst = nc.dram_tensor("pecst", [128, 296], f32, kind="ExternalInput").ap()
    vcst = nc.dram_tensor("vcst", [128, 52], f32, kind="ExternalInput").ap()
    y = nc.dram_tensor("y", [BLOC, 12], f32, kind="ExternalOutput").ap()
    with tile.TileContext(nc) as tc:
        with ExitStack() as ctx:
            xdma = _build_body(ctx, tc, x, pecst, vcst, y)
    _hoist_pre_barrier(nc, xdma)
    nc.compile()
    _NC_CACHE["nc"] = nc
    return nc


def _run(inputs_np, consts, trace=False):
    from concourse.bass_utils import run_bass_kernel_spmd
    nc = _get_nc()
    x = np.ascontiguousarray(np.asarray(inputs_np, np.float32))
    in_maps = []
    for c in range(NCORES):
        m = {"x": np.ascontiguousarray(x[BLOC * c:BLOC * (c + 1)])}
        m.update(consts)
        in_maps.append(m)
    res = run_bass_kernel_spmd(nc, in_maps, core_ids=list(range(NCORES)),
                               trace=trace)
    out = np.concatenate([r["y"] for r in res.results], axis=0)
    return out.astype(np.float32), res


def kernel(inputs, q_params, W1, b1, W2, b2):
    consts = _host_consts(q_params, W1, b1, W2, b2)
    out, _ = _run(inputs, consts, trace=False)
    return out
